# revision 2
# baseline (speedup 1.0000x reference)
"""Multi-head attention Trainium2 kernel v3 (8 NeuronCores, SPMD).

Problem: B=2, S=2048, d_model=1024, H=16 heads, dk=64.
    q = Q@WQ_h, k = K@WK_h, v = V@WV_h  (per head)
    scores = q k^T / sqrt(dk) + mask;  attn = softmax(scores)
    out = concat_h(attn @ v) @ WO

Sharding: 8 cores = 2 batches x 4 head-groups (4 heads each).  Each core
computes a full [S, d_model] partial output (its heads' contribution through
WO); host sums the 4 partials per batch.

All matmul inputs bf16 (fp8 was tried and measured 3-6% output error per
stage -- attention outputs are weighted averages, so per-element fp8 noise
does not dilute).  Dataflow per core:
  - host supplies transposed activations X^T [D, S]; q/k projected into
    [dk, S] layout (head pairs stacked on 128 partitions); v into
    [S, dk]+ones column
  - scores computed transposed: S^T[k, q] = k q^T (contraction dk=64, two
    heads row-packed at base partitions 0/64 -> concurrent on the PE)
  - attn_unnorm^T = exp(S^T/8) * exp(mask)^T  (exp on ScalarE PSUM->SBUF
    bf16 with scale=1/8; mask-mul is a 2x-rate bf16 TensorTensor on DVE)
  - PV: O^T[dk+1, q] = [v | 1]^T @ attn^T -- the ones column makes the
    softmax denominator Z[q] ride along as row 64
  - normalize O^T rows by 1/Z during PSUM eviction (reciprocal + gpsimd
    partition-broadcast of 1/Z)
  - WO: partial[q, n] accumulates lhsT = stacked O^T head-pair chunks
  - output bf16; host upcasts to f32 and sums the 4 partials per batch

Scheduling (the v3 part):
  - k/v activations stream in 512-column blocks with projections chasing
    the DMAs; first q-block's xq lands before xk so PE warms immediately
  - software pipeline across q blocks: next block's scores interleave with
    this block's PV/WO so ScalarE's exp queue never drains
  - q-proj eviction on ScalarE, everything else DVE, Z-broadcast on GpSimd
"""

import os
from contextlib import ExitStack

import numpy as np
import ml_dtypes

import concourse.bass as bass
import concourse.tile as tile
import concourse.mybir as mybir
from concourse import bacc
from concourse.bass_utils import run_bass_kernel_spmd

BF16 = mybir.dt.bfloat16
F32 = mybir.dt.float32

B = 2
S = 2048
D = 1024
H = 16
DK = 64
N_CORES = 8
HPC = H // (N_CORES // B)  # heads per core = 4
P = 128

NB_F = np.dtype(ml_dtypes.bfloat16)

# stash for test harness
LAST_RESULTS = None


def _build_program(repeat=1):
    nc = bacc.Bacc("TRN2", target_bir_lowering=False, debug=False)

    qT = nc.dram_tensor("qT", [D, S], BF16, kind="ExternalInput")
    kT = nc.dram_tensor("kT", [D, S], BF16, kind="ExternalInput")
    vT = nc.dram_tensor("vT", [D, S], BF16, kind="ExternalInput")
    eT = nc.dram_tensor("eT", [S, S], BF16, kind="ExternalInput")  # exp(mask)^T
    wq = nc.dram_tensor("wq", [D, HPC * DK], BF16, kind="ExternalInput")
    wk = nc.dram_tensor("wk", [D, HPC * DK], BF16, kind="ExternalInput")
    wv = nc.dram_tensor("wv", [D, HPC * DK], BF16, kind="ExternalInput")
    wo = nc.dram_tensor("wo", [HPC * DK, D], BF16, kind="ExternalInput")
    out = nc.dram_tensor("out", [S, D], BF16, kind="ExternalOutput")

    ND = D // P        # 8 contraction chunks of 128
    NK = S // P        # 16 key tiles
    NQ = S // 512      # 4 query blocks
    NPAIR = HPC // 2   # 2 head pairs

    with tile.TileContext(nc) as tc:
        with (
            tc.tile_pool(name="persist", bufs=1) as persist,
            tc.tile_pool(name="xq", bufs=2) as xq_pool,
            tc.tile_pool(name="xk", bufs=2) as xk_pool,
            tc.tile_pool(name="xv", bufs=2) as xv_pool,
            tc.tile_pool(name="eT_pool", bufs=2) as eT_pool,
            tc.tile_pool(name="es", bufs=4) as es_pool,
            tc.tile_pool(name="oT", bufs=4) as oT_pool,
            tc.tile_pool(name="rz", bufs=2) as rz_pool,
            tc.tile_pool(name="rzb", bufs=2) as rzb_pool,
            tc.tile_pool(name="outsb", bufs=2) as outsb_pool,
            tc.tile_pool(name="attn", bufs=2) as attn_pool,
            tc.tile_pool(name="ps_s", bufs=2, space="PSUM") as ps_s_pool,
            tc.tile_pool(name="ps_o", bufs=2, space="PSUM") as ps_o_pool,
            tc.tile_pool(name="ps_x", bufs=2, space="PSUM") as ps_x_pool,
        ):
            # ---- persistent SBUF ----
            w_sb = {}
            for name in ("wq", "wk", "wv"):
                w_sb[name] = persist.tile(
                    [P, ND, HPC * DK], BF16, tag=f"w_{name}", name=f"w_{name}"
                )
            wo_sb = persist.tile([P, NPAIR, D], BF16, tag="wo")

            qT_sb = persist.tile([P, NPAIR, S], BF16, tag="qT_sb")
            kT_sb = persist.tile([P, NPAIR, S], BF16, tag="kT_sb")
            v_sb = persist.tile([P, NK, HPC, DK + 1], BF16, tag="v_sb")


            prefetched = {}

            def prefetch(qb):
                if qb >= NQ or qb in prefetched:
                    return
                qs = slice(qb * 512, (qb + 1) * 512)
                xq_blk = xq_pool.tile(
                    [P, ND, 512], BF16, tag="xq_blk", name="xq_blk"
                )
                nc.sync.dma_start(
                    xq_blk, qT[:, qs].rearrange("(dc p) s -> p dc s", p=P)
                )
                eT_blk = eT_pool.tile(
                    [P, NK, 512], BF16, tag="eT_blk", name="eT_blk"
                )
                nc.sync.dma_start(
                    eT_blk, eT[:, qs].rearrange("(kc p) q -> p kc q", p=P)
                )
                prefetched[qb] = (xq_blk, eT_blk)

            def emit_qproj(qb, xq_blk):
                qs = slice(qb * 512, (qb + 1) * 512)
                for pr in range(NPAIR):
                    ps = ps_x_pool.tile([P, 512], F32, tag="ps_x", name="ps_q")
                    for dc in range(ND):
                        nc.tensor.matmul(
                            ps,
                            w_sb["wq"][:, dc, pr * P : (pr + 1) * P],
                            xq_blk[:, dc, :],
                            start=(dc == 0),
                            stop=(dc == ND - 1),
                        )
                    nc.vector.tensor_copy(qT_sb[:, pr, qs], ps)

            # ---- startup DMAs: wq/wk then qb0's xq, then xk blocks, then
            # qb0's mask, then the v side ----
            nc.sync.dma_start(w_sb["wq"], wq.rearrange("(dc p) m -> p dc m", p=P))
            nc.sync.dma_start(w_sb["wk"], wk.rearrange("(dc p) m -> p dc m", p=P))

            for _rep in range(repeat):
                xq_blk0 = xq_pool.tile([P, ND, 512], BF16, tag="xq_blk",
                                       name="xq_blk")
                nc.sync.dma_start(
                    xq_blk0, qT[:, 0:512].rearrange("(dc p) s -> p dc s", p=P)
                )
                eT_blk0 = eT_pool.tile([P, NK, 512], BF16, tag="eT_blk",
                                       name="eT_blk")
                prefetched[0] = (xq_blk0, eT_blk0)
                nc.sync.dma_start(
                    w_sb["wv"], wv.rearrange("(dc p) m -> p dc m", p=P)
                )
                nc.sync.dma_start(wo_sb, wo.rearrange("(pr p) n -> p pr n", p=P))
                nc.vector.memset(v_sb[:, :, :, DK : DK + 1], 1.0)

                emit_qproj(0, xq_blk0)

                # ---- k projection, double-buffered 512-column blocks;
                # qb0's mask streams in 4-key-tile chunks between the xk
                # blocks so the first mask-muls unblock progressively ----
                for sb in range(NQ):
                    ss = slice(sb * 512, (sb + 1) * 512)
                    xk_blk = xk_pool.tile([P, ND, 512], BF16, tag="xk",
                                          name="xk_blk")
                    nc.sync.dma_start(
                        xk_blk, kT[:, ss].rearrange("(dc p) s -> p dc s", p=P)
                    )
                    r0, r1 = sb * 4 * P, (sb + 1) * 4 * P
                    nc.sync.dma_start(
                        eT_blk0[:, sb * 4 : (sb + 1) * 4, :],
                        eT[r0:r1, 0:512].rearrange("(kc p) q -> p kc q", p=P),
                    )
                    for pr in range(NPAIR):
                        ps = ps_x_pool.tile([P, 512], F32, tag="ps_x", name="ps_k")
                        for dc in range(ND):
                            nc.tensor.matmul(
                                ps,
                                w_sb["wk"][:, dc, pr * P : (pr + 1) * P],
                                xk_blk[:, dc, :],
                                start=(dc == 0),
                                stop=(dc == ND - 1),
                            )
                        nc.vector.tensor_copy(kT_sb[:, pr, ss], ps)


                def emit_scores(qb, pr):
                    """Scores + exp + mask-mul for one head pair -> attnT."""
                    qs = slice(qb * 512, (qb + 1) * 512)
                    _, eT_blk = prefetched[qb]
                    attnT = [
                        attn_pool.tile(
                            [P, NK, 512], BF16,
                            tag=f"attnT{hh}", name=f"attnT{hh}",
                        )
                        for hh in range(2)
                    ]
                    for kg in range(NK // 2):
                        ps_sc = [
                            ps_s_pool.tile(
                                [P, 2, 512], F32, tag="ps_s", name=f"ps_sc{hh}"
                            )
                            for hh in range(2)
                        ]
                        # two heads row-packed (base partition 0/64) ->
                        # adjacent matmuls use distinct PE row groups and
                        # overlap on hardware
                        for i in range(2):
                            kc = kg * 2 + i
                            for hh in range(2):
                                hb = hh * DK
                                nc.tensor.matmul(
                                    ps_sc[hh][:, i, :],
                                    kT_sb[hb : hb + DK, pr, kc * P : (kc + 1) * P],
                                    qT_sb[hb : hb + DK, pr, qs],
                                    start=True,
                                    stop=True,
                                )
                        for hh in range(2):
                            es = es_pool.tile([P, 2, 512], BF16, tag="es")
                            nc.scalar.activation(
                                es,
                                ps_sc[hh],
                                mybir.ActivationFunctionType.Exp,
                                scale=0.125,
                            )
                            nc.vector.tensor_mul(
                                attnT[hh][:, kg * 2 : kg * 2 + 2, :],
                                es,
                                eT_blk[:, kg * 2 : kg * 2 + 2, :],
                            )
                    return attnT

                def emit_pv(qb, pr, attnT):
                    """PV for one head pair; ones column carries Z in row 64."""
                    oT_sb = oT_pool.tile([P, 512], BF16, tag="oT_sb")
                    for hh in range(2):
                        h = pr * 2 + hh
                        ps_o = ps_o_pool.tile(
                            [DK + 1, 512], F32, tag="ps_o", name="ps_o"
                        )
                        for kc in range(NK):
                            nc.tensor.matmul(
                                ps_o,
                                v_sb[:, kc, h, :],
                                attnT[hh][:, kc, :],
                                start=(kc == 0),
                                stop=(kc == NK - 1),
                            )
                        rz = rz_pool.tile([1, 512], F32, tag="rz")
                        nc.vector.reciprocal(rz, ps_o[DK : DK + 1, :])
                        rzb = rzb_pool.tile([DK, 512], F32, tag="rzb")
                        nc.gpsimd.partition_broadcast(rzb, rz)
                        nc.vector.tensor_mul(
                            oT_sb[hh * DK : (hh + 1) * DK, :],
                            ps_o[0:DK, :],
                            rzb,
                        )
                    return oT_sb

                def emit_wo(qb, oT_pair_sb):
                    # the last block's evictions go to ScalarE: its exp queue
                    # is empty by then, and DVE still has normalize work
                    last = qb == NQ - 1
                    for qq in range(4):
                        row0 = qb * 512 + qq * P
                        for nb in range(2):
                            ps_w = ps_x_pool.tile(
                                [P, 512], F32, tag="ps_x", name="ps_w"
                            )
                            for pr in range(NPAIR):
                                nc.tensor.matmul(
                                    ps_w,
                                    oT_pair_sb[pr][:, qq * P : (qq + 1) * P],
                                    wo_sb[:, pr, nb * 512 : (nb + 1) * 512],
                                    start=(pr == 0),
                                    stop=(pr == NPAIR - 1),
                                )
                            osb = outsb_pool.tile([P, 512], BF16, tag="osb")
                            if last:
                                nc.scalar.copy(osb, ps_w)
                            else:
                                nc.vector.tensor_copy(osb, ps_w)
                            nc.sync.dma_start(
                                out[row0 : row0 + P, nb * 512 : (nb + 1) * 512],
                                osb,
                            )

                # software pipeline across q blocks: the next block's scores
                # interleave with this block's PV so ScalarE's exp queue never
                # drains; WO is deferred one stage so PE never waits on the
                # DVE normalize chain
                sc0 = emit_scores(0, 0)

                # v projection here: PE filler while ScalarE drains pr0's
                # exps; also aligned with the xv block DMA arrivals
                # ---- v projection, double-buffered 512-column blocks ----
                for sb in range(NQ):
                    ss = slice(sb * 512, (sb + 1) * 512)
                    xv_blk = xv_pool.tile([P, ND, 512], BF16, tag="xv",
                                          name="xv_blk")
                    nc.sync.dma_start(
                        xv_blk, vT[:, ss].rearrange("(dc p) s -> p dc s", p=P)
                    )
                    for kk in range(4):
                        kc = sb * 4 + kk
                        ps = ps_x_pool.tile([P, HPC * DK], F32, tag="ps_x",
                                            name="ps_v")
                        for dc in range(ND):
                            nc.tensor.matmul(
                                ps,
                                xv_blk[:, dc, kk * P : (kk + 1) * P],
                                w_sb["wv"][:, dc, :],
                                start=(dc == 0),
                                stop=(dc == ND - 1),
                            )
                        nc.vector.tensor_copy(
                            v_sb[:, kc, :, 0:DK],
                            ps.rearrange("p (h j) -> p h j", h=HPC),
                        )

                saved_oT = None
                for qb in range(NQ):
                    sc1 = emit_scores(qb, 1)
                    if saved_oT is not None:
                        emit_wo(qb - 1, saved_oT)
                    if qb + 1 < NQ:
                        prefetch(qb + 1)
                        emit_qproj(qb + 1, prefetched[qb + 1][0])
                    oT0 = emit_pv(qb, 0, sc0)
                    if qb + 1 < NQ:
                        sc0 = emit_scores(qb + 1, 0)
                    oT1 = emit_pv(qb, 1, sc1)
                    saved_oT = [oT0, oT1]
                    prefetched.pop(qb)
                emit_wo(NQ - 1, saved_oT)

    nc.compile()
    return nc


_PROGRAM = None


def _get_program():
    global _PROGRAM
    if _PROGRAM is None:
        _PROGRAM = _build_program()
    return _PROGRAM


def prepare_in_maps(Q, K, V, additive_mask, WQ, WK, WV, WO):
    Q = np.asarray(Q, np.float32)
    K = np.asarray(K, np.float32)
    V = np.asarray(V, np.float32)
    mask = np.asarray(additive_mask, np.float32)
    WQ = np.asarray(WQ, np.float32)
    WK = np.asarray(WK, np.float32)
    WV = np.asarray(WV, np.float32)
    WO = np.asarray(WO, np.float32)

    # stacked weights [D, H*DK], head-major columns; the softmax 1/sqrt(dk)
    # is applied by the exp's scale=1/8
    wq_all = np.ascontiguousarray(WQ.transpose(1, 0, 2).reshape(D, H * DK))
    wk_all = np.ascontiguousarray(WK.transpose(1, 0, 2).reshape(D, H * DK))
    wv_all = np.ascontiguousarray(WV.transpose(1, 0, 2).reshape(D, H * DK))
    eT = np.ascontiguousarray(np.exp(mask).T).astype(NB_F)
    xT = {}
    for b in range(B):
        xT[("q", b)] = np.ascontiguousarray(Q[b].T).astype(NB_F)
        xT[("k", b)] = np.ascontiguousarray(K[b].T).astype(NB_F)
        xT[("v", b)] = np.ascontiguousarray(V[b].T).astype(NB_F)

    in_maps = []
    for c in range(N_CORES):
        b, g = divmod(c, N_CORES // B)
        hs = slice(g * HPC * DK, (g + 1) * HPC * DK)
        in_maps.append(
            {
                "qT": xT[("q", b)],
                "kT": xT[("k", b)],
                "vT": xT[("v", b)],
                "eT": eT,
                "wq": np.ascontiguousarray(wq_all[:, hs]).astype(NB_F),
                "wk": np.ascontiguousarray(wk_all[:, hs]).astype(NB_F),
                "wv": np.ascontiguousarray(wv_all[:, hs]).astype(NB_F),
                "wo": np.ascontiguousarray(WO[hs, :]).astype(NB_F),
            }
        )
    return in_maps


def kernel(Q, K, V, additive_mask, key_padding_mask, WQ, WK, WV, WO):
    global LAST_RESULTS
    in_maps = prepare_in_maps(Q, K, V, additive_mask, WQ, WK, WV, WO)
    nc = _get_program()
    res = run_bass_kernel_spmd(
        nc,
        in_maps,
        core_ids=list(range(N_CORES)),
        trace=False,
    )
    LAST_RESULTS = res

    full = np.zeros((B, S, D), np.float32)
    for c in range(N_CORES):
        b = c // (N_CORES // B)
        full[b] += np.asarray(res.results[c]["out"], dtype=np.float32)
    return full


# revision 4
# speedup vs baseline: 1.0010x; 1.0010x over previous
"""Multi-head attention Trainium2 kernel v3 (8 NeuronCores, SPMD).

Problem: B=2, S=2048, d_model=1024, H=16 heads, dk=64.
    q = Q@WQ_h, k = K@WK_h, v = V@WV_h  (per head)
    scores = q k^T / sqrt(dk) + mask;  attn = softmax(scores)
    out = concat_h(attn @ v) @ WO

Sharding: 8 cores = 2 batches x 4 head-groups (4 heads each).  Each core
computes a full [S, d_model] partial output (its heads' contribution through
WO); host sums the 4 partials per batch.

All matmul inputs bf16 (fp8 was tried and measured 3-6% output error per
stage -- attention outputs are weighted averages, so per-element fp8 noise
does not dilute).  Dataflow per core:
  - host supplies transposed activations X^T [D, S]; q/k projected into
    [dk, S] layout (head pairs stacked on 128 partitions); v into
    [S, dk]+ones column
  - scores computed transposed: S^T[k, q] = k q^T (contraction dk=64, two
    heads row-packed at base partitions 0/64 -> concurrent on the PE)
  - attn_unnorm^T = exp(S^T/8) * exp(mask)^T  (exp on ScalarE PSUM->SBUF
    bf16 with scale=1/8; mask-mul is a 2x-rate bf16 TensorTensor on DVE)
  - PV: O^T[dk+1, q] = [v | 1]^T @ attn^T -- the ones column makes the
    softmax denominator Z[q] ride along as row 64
  - normalize O^T rows by 1/Z during PSUM eviction (reciprocal + gpsimd
    partition-broadcast of 1/Z)
  - WO: partial[q, n] accumulates lhsT = stacked O^T head-pair chunks
  - output bf16; host upcasts to f32 and sums the 4 partials per batch

Scheduling (the v3 part):
  - k/v activations stream in 512-column blocks with projections chasing
    the DMAs; first q-block's xq lands before xk so PE warms immediately
  - software pipeline across q blocks: next block's scores interleave with
    this block's PV/WO so ScalarE's exp queue never drains
  - q-proj eviction on ScalarE, everything else DVE, Z-broadcast on GpSimd
"""

import os
from contextlib import ExitStack

import numpy as np
import ml_dtypes

import concourse.bass as bass
import concourse.tile as tile
import concourse.mybir as mybir
from concourse import bacc
from concourse.bass_utils import run_bass_kernel_spmd

BF16 = mybir.dt.bfloat16
F32 = mybir.dt.float32

B = 2
S = 2048
D = 1024
H = 16
DK = 64
N_CORES = 8
HPC = H // (N_CORES // B)  # heads per core = 4
P = 128

NB_F = np.dtype(ml_dtypes.bfloat16)

# stash for test harness
LAST_RESULTS = None


def _build_program(repeat=1):
    nc = bacc.Bacc("TRN2", target_bir_lowering=False, debug=False)

    ND = D // P        # 8 contraction chunks of 128
    NK = S // P        # 16 key tiles
    NQ = S // 512      # 4 query blocks
    NPAIR = HPC // 2   # 2 head pairs

    # all inputs host-pre-tiled: every DMA is a contiguous per-partition copy
    qT = nc.dram_tensor("qT", [NQ, P, ND, 512], BF16, kind="ExternalInput")
    kT = nc.dram_tensor("kT", [NQ, P, ND, 512], BF16, kind="ExternalInput")
    vT = nc.dram_tensor("vT", [NQ, P, ND, 512], BF16, kind="ExternalInput")
    eT = nc.dram_tensor("eT", [NQ, P, NK, 512], BF16, kind="ExternalInput")
    wq = nc.dram_tensor("wq", [P, ND, HPC * DK], BF16, kind="ExternalInput")
    wk = nc.dram_tensor("wk", [P, ND, HPC * DK], BF16, kind="ExternalInput")
    wv = nc.dram_tensor("wv", [P, ND, HPC * DK], BF16, kind="ExternalInput")
    wo = nc.dram_tensor("wo", [P, NPAIR, D], BF16, kind="ExternalInput")
    out = nc.dram_tensor("out", [S, D], BF16, kind="ExternalOutput")

    with tile.TileContext(nc) as tc:
        with (
            tc.tile_pool(name="persist", bufs=1) as persist,
            tc.tile_pool(name="xq", bufs=2) as xq_pool,
            tc.tile_pool(name="xk", bufs=2) as xk_pool,
            tc.tile_pool(name="xv", bufs=2) as xv_pool,
            tc.tile_pool(name="eT_pool", bufs=2) as eT_pool,
            tc.tile_pool(name="es", bufs=4) as es_pool,
            tc.tile_pool(name="oT", bufs=3) as oT_pool,
            tc.tile_pool(name="rz", bufs=2) as rz_pool,
            tc.tile_pool(name="rzb", bufs=2) as rzb_pool,
            tc.tile_pool(name="outsb", bufs=2) as outsb_pool,
            tc.tile_pool(name="attn", bufs=2) as attn_pool,
            tc.tile_pool(name="ps_s", bufs=2, space="PSUM") as ps_s_pool,
            tc.tile_pool(name="ps_o", bufs=2, space="PSUM") as ps_o_pool,
            tc.tile_pool(name="ps_x", bufs=2, space="PSUM") as ps_x_pool,
        ):
            # ---- persistent SBUF ----
            w_sb = {}
            for name in ("wq", "wk", "wv"):
                w_sb[name] = persist.tile(
                    [P, ND, HPC * DK], BF16, tag=f"w_{name}", name=f"w_{name}"
                )
            wo_sb = persist.tile([P, NPAIR, D], BF16, tag="wo")

            qT_sb = persist.tile([P, NPAIR, S], BF16, tag="qT_sb")
            kT_sb = persist.tile([P, NPAIR, S], BF16, tag="kT_sb")
            v_sb = persist.tile([P, NK, HPC, DK + 1], BF16, tag="v_sb")


            prefetched = {}

            def prefetch_xq(qb):
                if qb >= NQ or qb in prefetched:
                    return
                xq_blk = xq_pool.tile(
                    [P, ND, 512], BF16, tag="xq_blk", name="xq_blk"
                )
                nc.sync.dma_start(xq_blk, qT[qb, :, :, :])
                prefetched[qb] = (xq_blk, None)

            def prefetch(qb):
                if qb >= NQ:
                    return
                prefetch_xq(qb)
                if prefetched[qb][1] is None:
                    eT_blk = eT_pool.tile(
                        [P, NK, 512], BF16, tag="eT_blk", name="eT_blk"
                    )
                    nc.sync.dma_start(eT_blk, eT[qb, :, :, :])
                    prefetched[qb] = (prefetched[qb][0], eT_blk)

            def emit_qproj(qb, xq_blk):
                qs = slice(qb * 512, (qb + 1) * 512)
                for pr in range(NPAIR):
                    ps = ps_x_pool.tile([P, 512], F32, tag="ps_x", name="ps_q")
                    for dc in range(ND):
                        nc.tensor.matmul(
                            ps,
                            w_sb["wq"][:, dc, pr * P : (pr + 1) * P],
                            xq_blk[:, dc, :],
                            start=(dc == 0),
                            stop=(dc == ND - 1),
                        )
                    nc.vector.tensor_copy(qT_sb[:, pr, qs], ps)

            # ---- startup DMAs: wq/wk then qb0's xq, then xk blocks, then
            # qb0's mask, then the v side ----
            nc.sync.dma_start(w_sb["wq"], wq[:, :, :])

            for _rep in range(repeat):
                xq_blk0 = xq_pool.tile([P, ND, 512], BF16, tag="xq_blk",
                                       name="xq_blk")
                nc.sync.dma_start(xq_blk0, qT[0, :, :, :])
                nc.sync.dma_start(w_sb["wk"], wk[:, :, :])
                eT_blk0 = eT_pool.tile([P, NK, 512], BF16, tag="eT_blk",
                                       name="eT_blk")
                prefetched[0] = (xq_blk0, eT_blk0)
                nc.vector.memset(v_sb[:, :, :, DK : DK + 1], 1.0)

                emit_qproj(0, xq_blk0)

                def emit_scores(qb, pr, kg0=0, kg1=NK // 2, attnT=None):
                    """Scores + exp + mask-mul for one head pair -> attnT."""
                    qs = slice(qb * 512, (qb + 1) * 512)
                    _, eT_blk = prefetched[qb]
                    if attnT is None:
                        attnT = [
                            attn_pool.tile(
                                [P, NK, 512], BF16,
                                tag=f"attnT{hh}", name=f"attnT{hh}",
                            )
                            for hh in range(2)
                        ]
                    for kg in range(kg0, kg1):
                        ps_sc = [
                            ps_s_pool.tile(
                                [P, 2, 512], F32, tag="ps_s", name=f"ps_sc{hh}"
                            )
                            for hh in range(2)
                        ]
                        # two heads row-packed (base partition 0/64) ->
                        # adjacent matmuls use distinct PE row groups and
                        # overlap on hardware
                        for i in range(2):
                            kc = kg * 2 + i
                            for hh in range(2):
                                hb = hh * DK
                                nc.tensor.matmul(
                                    ps_sc[hh][:, i, :],
                                    kT_sb[hb : hb + DK, pr, kc * P : (kc + 1) * P],
                                    qT_sb[hb : hb + DK, pr, qs],
                                    start=True,
                                    stop=True,
                                )
                        for hh in range(2):
                            es = es_pool.tile([P, 2, 512], BF16, tag="es")
                            nc.scalar.activation(
                                es,
                                ps_sc[hh],
                                mybir.ActivationFunctionType.Exp,
                                scale=0.125,
                            )
                            nc.vector.tensor_mul(
                                attnT[hh][:, kg * 2 : kg * 2 + 2, :],
                                es,
                                eT_blk[:, kg * 2 : kg * 2 + 2, :],
                            )
                    return attnT


                # ---- k projection, double-buffered 512-column blocks;
                # qb0's mask streams in 4-key-tile chunks between the xk
                # blocks, and qb0/pr0's score groups chase the k blocks so
                # ScalarE's exp pipeline starts as early as possible ----
                sc0 = None
                for sb in range(NQ):
                    ss = slice(sb * 512, (sb + 1) * 512)
                    xk_blk = xk_pool.tile([P, ND, 512], BF16, tag="xk",
                                          name="xk_blk")
                    nc.sync.dma_start(xk_blk, kT[sb, :, :, :])
                    nc.sync.dma_start(
                        eT_blk0[:, sb * 4 : (sb + 1) * 4, :],
                        eT[0, :, sb * 4 : (sb + 1) * 4, :],
                    )
                    for pr in range(NPAIR):
                        ps = ps_x_pool.tile([P, 512], F32, tag="ps_x", name="ps_k")
                        for dc in range(ND):
                            nc.tensor.matmul(
                                ps,
                                w_sb["wk"][:, dc, pr * P : (pr + 1) * P],
                                xk_blk[:, dc, :],
                                start=(dc == 0),
                                stop=(dc == ND - 1),
                            )
                        nc.vector.tensor_copy(kT_sb[:, pr, ss], ps)
                    sc0 = emit_scores(0, 0, 2 * sb, 2 * sb + 2, sc0)


                def emit_pv(qb, pr, attnT):
                    """PV for one head pair; ones column carries Z in row 64."""
                    oT_sb = oT_pool.tile([P, 512], BF16, tag="oT_sb")
                    for hh in range(2):
                        h = pr * 2 + hh
                        ps_o = ps_o_pool.tile(
                            [DK + 1, 512], F32, tag="ps_o", name="ps_o"
                        )
                        for kc in range(NK):
                            nc.tensor.matmul(
                                ps_o,
                                v_sb[:, kc, h, :],
                                attnT[hh][:, kc, :],
                                start=(kc == 0),
                                stop=(kc == NK - 1),
                            )
                        rz = rz_pool.tile([1, 512], F32, tag="rz")
                        nc.vector.reciprocal(rz, ps_o[DK : DK + 1, :])
                        rzb = rzb_pool.tile([DK, 512], F32, tag="rzb")
                        nc.gpsimd.partition_broadcast(rzb, rz)
                        nc.vector.tensor_mul(
                            oT_sb[hh * DK : (hh + 1) * DK, :],
                            ps_o[0:DK, :],
                            rzb,
                        )
                    return oT_sb

                def emit_wo(qb, oT_pair_sb):
                    # the last block's evictions go to ScalarE (its exp queue
                    # is empty by then); both 512-col halves share one osb so
                    # each row block ships as a single 256KB DMA
                    last = qb == NQ - 1
                    for qq in range(4):
                        row0 = qb * 512 + qq * P
                        osb = outsb_pool.tile([P, 2, 512], BF16, tag="osb")
                        for nb in range(2):
                            ps_w = ps_x_pool.tile(
                                [P, 512], F32, tag="ps_x", name="ps_w"
                            )
                            for pr in range(NPAIR):
                                nc.tensor.matmul(
                                    ps_w,
                                    oT_pair_sb[pr][:, qq * P : (qq + 1) * P],
                                    wo_sb[:, pr, nb * 512 : (nb + 1) * 512],
                                    start=(pr == 0),
                                    stop=(pr == NPAIR - 1),
                                )
                            if last:
                                nc.scalar.copy(osb[:, nb, :], ps_w)
                            else:
                                nc.vector.tensor_copy(osb[:, nb, :], ps_w)
                        nc.sync.dma_start(
                            out[row0 : row0 + P, :],
                            osb.rearrange("p a q -> p (a q)"),
                        )

                # software pipeline across q blocks: the next block's scores
                # interleave with this block's PV so ScalarE's exp queue never
                # drains; WO is deferred one stage so PE never waits on the
                # DVE normalize chain
                # ---- v projection interleaved with qb0/pr1's score
                # groups: ScalarE keeps a full exp queue while PE fills v ----
                nc.sync.dma_start(w_sb["wv"], wv[:, :, :])
                sc1_0 = None
                for sb in range(NQ):
                    sc1_0 = emit_scores(0, 1, 2 * sb, 2 * sb + 2, sc1_0)
                    xv_blk = xv_pool.tile([P, ND, 512], BF16, tag="xv",
                                          name="xv_blk")
                    nc.sync.dma_start(xv_blk, vT[sb, :, :, :])
                    if sb == 0:
                        # qb1's xq jumps the queue so the next q-block's
                        # projection isn't starved behind xv; its mask
                        # follows after the last xv block
                        prefetch_xq(1)
                    if sb == NQ - 1:
                        prefetch(1)
                    for kk in range(4):
                        kc = sb * 4 + kk
                        ps = ps_x_pool.tile([P, HPC * DK], F32, tag="ps_x",
                                            name="ps_v")
                        for dc in range(ND):
                            nc.tensor.matmul(
                                ps,
                                xv_blk[:, dc, kk * P : (kk + 1) * P],
                                w_sb["wv"][:, dc, :],
                                start=(dc == 0),
                                stop=(dc == ND - 1),
                            )
                        nc.vector.tensor_copy(
                            v_sb[:, kc, :, 0:DK],
                            ps.rearrange("p (h j) -> p h j", h=HPC),
                        )
                nc.sync.dma_start(wo_sb, wo[:, :, :])

                saved_oT = None
                for qb in range(NQ):
                    sc1 = sc1_0 if qb == 0 else emit_scores(qb, 1)
                    if saved_oT is not None:
                        emit_wo(qb - 1, saved_oT)
                    if qb + 1 < NQ:
                        prefetch(qb + 1)
                        emit_qproj(qb + 1, prefetched[qb + 1][0])
                    oT0 = emit_pv(qb, 0, sc0)
                    if qb + 1 < NQ:
                        sc0 = emit_scores(qb + 1, 0)
                    oT1 = emit_pv(qb, 1, sc1)
                    saved_oT = [oT0, oT1]
                    prefetched.pop(qb)
                emit_wo(NQ - 1, saved_oT)

    nc.compile()
    return nc


_PROGRAM = None


def _get_program():
    global _PROGRAM
    if _PROGRAM is None:
        _PROGRAM = _build_program()
    return _PROGRAM


def prepare_in_maps(Q, K, V, additive_mask, WQ, WK, WV, WO):
    Q = np.asarray(Q, np.float32)
    K = np.asarray(K, np.float32)
    V = np.asarray(V, np.float32)
    mask = np.asarray(additive_mask, np.float32)
    WQ = np.asarray(WQ, np.float32)
    WK = np.asarray(WK, np.float32)
    WV = np.asarray(WV, np.float32)
    WO = np.asarray(WO, np.float32)

    ND, NK, NQ, NPAIR = D // P, S // P, S // 512, HPC // 2

    def tile_x(xT_mat):
        # [D, S] -> [NQ, P, ND, 512] with d = dc*128 + p, s = sb*512 + j
        return np.ascontiguousarray(
            xT_mat.reshape(ND, P, NQ, 512).transpose(2, 1, 0, 3)
        ).astype(NB_F)

    def tile_w(w_cols):
        # [D, M] -> [P, ND, M]
        M = w_cols.shape[1]
        return np.ascontiguousarray(
            w_cols.reshape(ND, P, M).transpose(1, 0, 2)
        ).astype(NB_F)

    # stacked weights head-major; the softmax 1/sqrt(dk) is applied by the
    # exp's scale=1/8
    wq_all = WQ.transpose(1, 0, 2).reshape(D, H * DK)
    wk_all = WK.transpose(1, 0, 2).reshape(D, H * DK)
    wv_all = WV.transpose(1, 0, 2).reshape(D, H * DK)
    eT = np.ascontiguousarray(
        np.exp(mask).T.reshape(NK, P, NQ, 512).transpose(2, 1, 0, 3)
    ).astype(NB_F)
    xT = {}
    for b in range(B):
        xT[("q", b)] = tile_x(Q[b].T)
        xT[("k", b)] = tile_x(K[b].T)
        xT[("v", b)] = tile_x(V[b].T)

    in_maps = []
    for c in range(N_CORES):
        b, g = divmod(c, N_CORES // B)
        hs = slice(g * HPC * DK, (g + 1) * HPC * DK)
        wo_t = np.ascontiguousarray(
            WO[hs, :].reshape(NPAIR, P, D).transpose(1, 0, 2)
        ).astype(NB_F)
        in_maps.append(
            {
                "qT": xT[("q", b)],
                "kT": xT[("k", b)],
                "vT": xT[("v", b)],
                "eT": eT,
                "wq": tile_w(wq_all[:, hs]),
                "wk": tile_w(wk_all[:, hs]),
                "wv": tile_w(wv_all[:, hs]),
                "wo": wo_t,
            }
        )
    return in_maps


def kernel(Q, K, V, additive_mask, key_padding_mask, WQ, WK, WV, WO):
    global LAST_RESULTS
    in_maps = prepare_in_maps(Q, K, V, additive_mask, WQ, WK, WV, WO)
    nc = _get_program()
    res = run_bass_kernel_spmd(
        nc,
        in_maps,
        core_ids=list(range(N_CORES)),
        trace=False,
    )
    LAST_RESULTS = res

    full = np.zeros((B, S, D), np.float32)
    for c in range(N_CORES):
        b = c // (N_CORES // B)
        full[b] += np.asarray(res.results[c]["out"], dtype=np.float32)
    return full


# revision 5
# speedup vs baseline: 1.1608x; 1.1597x over previous
"""Multi-head attention Trainium2 kernel v3 (8 NeuronCores, SPMD).

Problem: B=2, S=2048, d_model=1024, H=16 heads, dk=64.
    q = Q@WQ_h, k = K@WK_h, v = V@WV_h  (per head)
    scores = q k^T / sqrt(dk) + mask;  attn = softmax(scores)
    out = concat_h(attn @ v) @ WO

Sharding: 8 cores = 2 batches x 4 head-groups (4 heads each).  Each core
computes a full [S, d_model] partial output (its heads' contribution through
WO); host sums the 4 partials per batch.

All matmul inputs bf16 (fp8 was tried and measured 3-6% output error per
stage -- attention outputs are weighted averages, so per-element fp8 noise
does not dilute).  Dataflow per core:
  - host supplies transposed activations X^T [D, S]; q/k projected into
    [dk, S] layout (head pairs stacked on 128 partitions); v into
    [S, dk]+ones column
  - scores computed transposed: S^T[k, q] = k q^T (contraction dk=64, two
    heads row-packed at base partitions 0/64 -> concurrent on the PE)
  - attn_unnorm^T = exp(S^T/8) * exp(mask)^T  (exp on ScalarE PSUM->SBUF
    bf16 with scale=1/8; mask-mul is a 2x-rate bf16 TensorTensor on DVE)
  - PV: O^T[dk+1, q] = [v | 1]^T @ attn^T -- the ones column makes the
    softmax denominator Z[q] ride along as row 64
  - normalize O^T rows by 1/Z during PSUM eviction (reciprocal + gpsimd
    partition-broadcast of 1/Z)
  - WO: partial[q, n] accumulates lhsT = stacked O^T head-pair chunks
  - output bf16; host upcasts to f32 and sums the 4 partials per batch

Scheduling (the v3 part):
  - k/v activations stream in 512-column blocks with projections chasing
    the DMAs; first q-block's xq lands before xk so PE warms immediately
  - software pipeline across q blocks: next block's scores interleave with
    this block's PV/WO so ScalarE's exp queue never drains
  - q-proj eviction on ScalarE, everything else DVE, Z-broadcast on GpSimd
"""

import os
from contextlib import ExitStack

import numpy as np
import ml_dtypes

import concourse.bass as bass
import concourse.tile as tile
import concourse.mybir as mybir
from concourse import bacc
from concourse.bass_utils import run_bass_kernel_spmd

BF16 = mybir.dt.bfloat16
F32 = mybir.dt.float32

B = 2
S = 2048
D = 1024
H = 16
DK = 64
N_CORES = 8
HPC = H // (N_CORES // B)  # heads per core = 4
P = 128

NB_F = np.dtype(ml_dtypes.bfloat16)

# stash for test harness
LAST_RESULTS = None


def _build_program(repeat=1):
    nc = bacc.Bacc("TRN2", target_bir_lowering=False, debug=False)

    ND = D // P        # 8 contraction chunks of 128
    NK = S // P        # 16 key tiles
    NQ = S // 512      # 4 query blocks
    NPAIR = HPC // 2   # 2 head pairs

    # all inputs host-pre-tiled: every DMA is a contiguous per-partition copy
    qT = nc.dram_tensor("qT", [NQ, P, ND, 512], BF16, kind="ExternalInput")
    kT = nc.dram_tensor("kT", [NQ, P, ND, 512], BF16, kind="ExternalInput")
    vT = nc.dram_tensor("vT", [NQ, P, ND, 512], BF16, kind="ExternalInput")
    eT = nc.dram_tensor("eT", [NQ, P, NK, 512], BF16, kind="ExternalInput")
    wq = nc.dram_tensor("wq", [P, ND, HPC * DK], BF16, kind="ExternalInput")
    wk = nc.dram_tensor("wk", [P, ND, HPC * DK], BF16, kind="ExternalInput")
    wv = nc.dram_tensor("wv", [P, ND, HPC * DK], BF16, kind="ExternalInput")
    wo = nc.dram_tensor("wo", [P, NPAIR, D], BF16, kind="ExternalInput")
    out = nc.dram_tensor("out", [S, D], BF16, kind="ExternalOutput")

    with tile.TileContext(nc) as tc:
        with (
            tc.tile_pool(name="persist", bufs=1) as persist,
            tc.tile_pool(name="xq", bufs=2) as xq_pool,
            tc.tile_pool(name="xk", bufs=2) as xk_pool,
            tc.tile_pool(name="xv", bufs=2) as xv_pool,
            tc.tile_pool(name="eT_pool", bufs=2) as eT_pool,
            tc.tile_pool(name="es", bufs=4) as es_pool,
            tc.tile_pool(name="oT", bufs=3) as oT_pool,
            tc.tile_pool(name="rz", bufs=2) as rz_pool,
            tc.tile_pool(name="rzb", bufs=2) as rzb_pool,
            tc.tile_pool(name="outsb", bufs=2) as outsb_pool,
            tc.tile_pool(name="attn", bufs=2) as attn_pool,
            tc.tile_pool(name="ps_s", bufs=2, space="PSUM") as ps_s_pool,
            tc.tile_pool(name="ps_o", bufs=2, space="PSUM") as ps_o_pool,
            tc.tile_pool(name="ps_x", bufs=2, space="PSUM") as ps_x_pool,
        ):
            # ---- persistent SBUF ----
            w_sb = {}
            for name in ("wq", "wk", "wv"):
                w_sb[name] = persist.tile(
                    [P, ND, HPC * DK], BF16, tag=f"w_{name}", name=f"w_{name}"
                )
            wo_sb = persist.tile([P, NPAIR, D], BF16, tag="wo")

            qT_sb = persist.tile([P, NPAIR, S], BF16, tag="qT_sb")
            kT_sb = persist.tile([P, NPAIR, S], BF16, tag="kT_sb")
            v_sb = persist.tile([P, NK, HPC, DK + 1], BF16, tag="v_sb")


            prefetched = {}

            def prefetch_xq(qb):
                if qb >= NQ or qb in prefetched:
                    return
                xq_blk = xq_pool.tile(
                    [P, ND, 512], BF16, tag="xq_blk", name="xq_blk"
                )
                nc.sync.dma_start(xq_blk, qT[qb, :, :, :])
                prefetched[qb] = (xq_blk, None)

            def prefetch(qb):
                if qb >= NQ:
                    return
                prefetch_xq(qb)
                if prefetched[qb][1] is None:
                    eT_blk = eT_pool.tile(
                        [P, NK, 512], BF16, tag="eT_blk", name="eT_blk"
                    )
                    nc.sync.dma_start(eT_blk, eT[qb, :, :, :])
                    prefetched[qb] = (prefetched[qb][0], eT_blk)

            def emit_qproj(qb, xq_blk):
                qs = slice(qb * 512, (qb + 1) * 512)
                for pr in range(NPAIR):
                    ps = ps_x_pool.tile([P, 512], F32, tag="ps_x", name="ps_q")
                    for dc in range(ND):
                        nc.tensor.matmul(
                            ps,
                            w_sb["wq"][:, dc, pr * P : (pr + 1) * P],
                            xq_blk[:, dc, :],
                            start=(dc == 0),
                            stop=(dc == ND - 1),
                        )
                    nc.vector.tensor_copy(qT_sb[:, pr, qs], ps)

            # ---- startup DMAs: wq/wk then qb0's xq, then xk blocks, then
            # qb0's mask, then the v side ----
            nc.sync.dma_start(w_sb["wq"], wq[:, :, :])

            for _rep in range(repeat):
                xq_blk0 = xq_pool.tile([P, ND, 512], BF16, tag="xq_blk",
                                       name="xq_blk")
                nc.sync.dma_start(xq_blk0, qT[0, :, :, :])
                nc.sync.dma_start(w_sb["wk"], wk[:, :, :])
                eT_blk0 = eT_pool.tile([P, NK, 512], BF16, tag="eT_blk",
                                       name="eT_blk")
                prefetched[0] = (xq_blk0, eT_blk0)
                nc.vector.memset(v_sb[:, :, :, DK : DK + 1], 1.0)

                emit_qproj(0, xq_blk0)

                def emit_scores(qb, pr, kg0=0, kg1=NK // 2, attnT=None):
                    """Scores + exp + mask-mul for one head pair -> attnT."""
                    qs = slice(qb * 512, (qb + 1) * 512)
                    _, eT_blk = prefetched[qb]
                    if attnT is None:
                        attnT = [
                            attn_pool.tile(
                                [P, NK, 512], BF16,
                                tag=f"attnT{hh}", name=f"attnT{hh}",
                            )
                            for hh in range(2)
                        ]
                    for kg in range(kg0, kg1):
                        ps_sc = [
                            ps_s_pool.tile(
                                [P, 2, 512], F32, tag="ps_s", name=f"ps_sc{hh}"
                            )
                            for hh in range(2)
                        ]
                        # two heads row-packed (base partition 0/64) ->
                        # adjacent matmuls use distinct PE row groups and
                        # overlap on hardware
                        for i in range(2):
                            kc = kg * 2 + i
                            for hh in range(2):
                                hb = hh * DK
                                nc.tensor.matmul(
                                    ps_sc[hh][:, i, :],
                                    kT_sb[hb : hb + DK, pr, kc * P : (kc + 1) * P],
                                    qT_sb[hb : hb + DK, pr, qs],
                                    start=True,
                                    stop=True,
                                )
                        for hh in range(2):
                            es = es_pool.tile([P, 2, 512], BF16, tag="es")
                            nc.scalar.activation(
                                es,
                                ps_sc[hh],
                                mybir.ActivationFunctionType.Exp,
                                scale=0.125,
                            )
                            nc.vector.tensor_mul(
                                attnT[hh][:, kg * 2 : kg * 2 + 2, :],
                                es,
                                eT_blk[:, kg * 2 : kg * 2 + 2, :],
                            )
                    return attnT


                # ---- k projection, double-buffered 512-column blocks;
                # qb0's mask streams in 4-key-tile chunks between the xk
                # blocks, and qb0/pr0's score groups chase the k blocks so
                # ScalarE's exp pipeline starts as early as possible ----
                sc0 = None
                for sb in range(NQ):
                    ss = slice(sb * 512, (sb + 1) * 512)
                    xk_blk = xk_pool.tile([P, ND, 512], BF16, tag="xk",
                                          name="xk_blk")
                    nc.sync.dma_start(xk_blk, kT[sb, :, :, :])
                    nc.sync.dma_start(
                        eT_blk0[:, sb * 4 : (sb + 1) * 4, :],
                        eT[0, :, sb * 4 : (sb + 1) * 4, :],
                    )
                    for pr in range(NPAIR):
                        ps = ps_x_pool.tile([P, 512], F32, tag="ps_x", name="ps_k")
                        for dc in range(ND):
                            nc.tensor.matmul(
                                ps,
                                w_sb["wk"][:, dc, pr * P : (pr + 1) * P],
                                xk_blk[:, dc, :],
                                start=(dc == 0),
                                stop=(dc == ND - 1),
                            )
                        nc.vector.tensor_copy(kT_sb[:, pr, ss], ps)
                    sc0 = emit_scores(0, 0, 2 * sb, 2 * sb + 2, sc0)


                def emit_pv(qb, pr, attnT):
                    """PV for one head pair; ones column carries Z in row 64."""
                    oT_sb = oT_pool.tile([P, 512], BF16, tag="oT_sb")
                    for hh in range(2):
                        h = pr * 2 + hh
                        ps_o = ps_o_pool.tile(
                            [DK + 1, 512], F32, tag="ps_o", name="ps_o"
                        )
                        for kc in range(NK):
                            nc.tensor.matmul(
                                ps_o,
                                v_sb[:, kc, h, :],
                                attnT[hh][:, kc, :],
                                start=(kc == 0),
                                stop=(kc == NK - 1),
                            )
                        rz = rz_pool.tile([1, 512], F32, tag="rz")
                        nc.vector.reciprocal(rz, ps_o[DK : DK + 1, :])
                        rzb = rzb_pool.tile([DK, 512], F32, tag="rzb")
                        nc.gpsimd.partition_broadcast(rzb, rz)
                        nc.vector.tensor_mul(
                            oT_sb[hh * DK : (hh + 1) * DK, :],
                            ps_o[0:DK, :],
                            rzb,
                        )
                    return oT_sb

                def emit_wo(qb, oT_pair_sb):
                    # both 512-col halves share one osb so each row block
                    # ships as a single 256KB DMA; on the last block the
                    # evictions alternate ScalarE/DVE (both idle at the tail)
                    last = qb == NQ - 1
                    for qq in range(4):
                        row0 = qb * 512 + qq * P
                        osb = outsb_pool.tile([P, 2, 512], BF16, tag="osb")
                        for nb in range(2):
                            ps_w = ps_x_pool.tile(
                                [P, 512], F32, tag="ps_x", name="ps_w"
                            )
                            for pr in range(NPAIR):
                                nc.tensor.matmul(
                                    ps_w,
                                    oT_pair_sb[pr][:, qq * P : (qq + 1) * P],
                                    wo_sb[:, pr, nb * 512 : (nb + 1) * 512],
                                    start=(pr == 0),
                                    stop=(pr == NPAIR - 1),
                                )
                            if last and nb == 0:
                                nc.scalar.copy(osb[:, nb, :], ps_w)
                            else:
                                nc.vector.tensor_copy(osb[:, nb, :], ps_w)
                        nc.sync.dma_start(
                            out[row0 : row0 + P, :],
                            osb.rearrange("p a q -> p (a q)"),
                        )

                # software pipeline across q blocks: the next block's scores
                # interleave with this block's PV so ScalarE's exp queue never
                # drains; WO is deferred one stage so PE never waits on the
                # DVE normalize chain
                # ---- v projection interleaved with qb0/pr1's score
                # groups: ScalarE keeps a full exp queue while PE fills v ----
                nc.sync.dma_start(w_sb["wv"], wv[:, :, :])
                sc1_0 = None
                for sb in range(NQ):
                    sc1_0 = emit_scores(0, 1, 2 * sb, 2 * sb + 2, sc1_0)
                    xv_blk = xv_pool.tile([P, ND, 512], BF16, tag="xv",
                                          name="xv_blk")
                    nc.sync.dma_start(xv_blk, vT[sb, :, :, :])
                    if sb == 0:
                        # qb1's xq jumps the queue so the next q-block's
                        # projection isn't starved behind xv; its mask
                        # follows after the last xv block
                        prefetch_xq(1)
                    if sb == NQ - 1:
                        prefetch(1)
                    for kk in range(4):
                        kc = sb * 4 + kk
                        ps = ps_x_pool.tile([P, HPC * DK], F32, tag="ps_x",
                                            name="ps_v")
                        for dc in range(ND):
                            nc.tensor.matmul(
                                ps,
                                xv_blk[:, dc, kk * P : (kk + 1) * P],
                                w_sb["wv"][:, dc, :],
                                start=(dc == 0),
                                stop=(dc == ND - 1),
                            )
                        nc.vector.tensor_copy(
                            v_sb[:, kc, :, 0:DK],
                            ps.rearrange("p (h j) -> p h j", h=HPC),
                        )
                nc.sync.dma_start(wo_sb, wo[:, :, :])

                saved_oT = None
                for qb in range(NQ):
                    sc1 = sc1_0 if qb == 0 else emit_scores(qb, 1)
                    if saved_oT is not None:
                        emit_wo(qb - 1, saved_oT)
                    if qb + 1 < NQ:
                        prefetch(qb + 1)
                        emit_qproj(qb + 1, prefetched[qb + 1][0])
                    oT0 = emit_pv(qb, 0, sc0)
                    if qb + 1 < NQ:
                        sc0 = emit_scores(qb + 1, 0)
                    oT1 = emit_pv(qb, 1, sc1)
                    saved_oT = [oT0, oT1]
                    prefetched.pop(qb)
                emit_wo(NQ - 1, saved_oT)

    nc.compile()
    return nc


_PROGRAM = None


def _get_program():
    global _PROGRAM
    if _PROGRAM is None:
        _PROGRAM = _build_program()
    return _PROGRAM


def prepare_in_maps(Q, K, V, additive_mask, WQ, WK, WV, WO):
    Q = np.asarray(Q, np.float32)
    K = np.asarray(K, np.float32)
    V = np.asarray(V, np.float32)
    mask = np.asarray(additive_mask, np.float32)
    WQ = np.asarray(WQ, np.float32)
    WK = np.asarray(WK, np.float32)
    WV = np.asarray(WV, np.float32)
    WO = np.asarray(WO, np.float32)

    ND, NK, NQ, NPAIR = D // P, S // P, S // 512, HPC // 2

    def tile_x(xT_mat):
        # [D, S] -> [NQ, P, ND, 512] with d = dc*128 + p, s = sb*512 + j
        return np.ascontiguousarray(
            xT_mat.reshape(ND, P, NQ, 512).transpose(2, 1, 0, 3)
        ).astype(NB_F)

    def tile_w(w_cols):
        # [D, M] -> [P, ND, M]
        M = w_cols.shape[1]
        return np.ascontiguousarray(
            w_cols.reshape(ND, P, M).transpose(1, 0, 2)
        ).astype(NB_F)

    # stacked weights head-major; the softmax 1/sqrt(dk) is applied by the
    # exp's scale=1/8
    wq_all = WQ.transpose(1, 0, 2).reshape(D, H * DK)
    wk_all = WK.transpose(1, 0, 2).reshape(D, H * DK)
    wv_all = WV.transpose(1, 0, 2).reshape(D, H * DK)
    eT = np.ascontiguousarray(
        np.exp(mask).T.reshape(NK, P, NQ, 512).transpose(2, 1, 0, 3)
    ).astype(NB_F)
    xT = {}
    for b in range(B):
        xT[("q", b)] = tile_x(Q[b].T)
        xT[("k", b)] = tile_x(K[b].T)
        xT[("v", b)] = tile_x(V[b].T)

    in_maps = []
    for c in range(N_CORES):
        b, g = divmod(c, N_CORES // B)
        hs = slice(g * HPC * DK, (g + 1) * HPC * DK)
        wo_t = np.ascontiguousarray(
            WO[hs, :].reshape(NPAIR, P, D).transpose(1, 0, 2)
        ).astype(NB_F)
        in_maps.append(
            {
                "qT": xT[("q", b)],
                "kT": xT[("k", b)],
                "vT": xT[("v", b)],
                "eT": eT,
                "wq": tile_w(wq_all[:, hs]),
                "wk": tile_w(wk_all[:, hs]),
                "wv": tile_w(wv_all[:, hs]),
                "wo": wo_t,
            }
        )
    return in_maps


def kernel(Q, K, V, additive_mask, key_padding_mask, WQ, WK, WV, WO):
    global LAST_RESULTS
    in_maps = prepare_in_maps(Q, K, V, additive_mask, WQ, WK, WV, WO)
    nc = _get_program()
    res = run_bass_kernel_spmd(
        nc,
        in_maps,
        core_ids=list(range(N_CORES)),
        trace=False,
    )
    LAST_RESULTS = res

    full = np.zeros((B, S, D), np.float32)
    for c in range(N_CORES):
        b = c // (N_CORES // B)
        full[b] += np.asarray(res.results[c]["out"], dtype=np.float32)
    return full
